# revision 32
# baseline (speedup 1.0000x reference)
"""Causal self-attention (B=2, L=2048, D=1024, H=16) on 8 trn2 NeuronCores.

Sharding: core c = 4*b + g handles batch b and head group g (4 heads).
Per core: QKV projection for its heads' weight columns (tensor-parallel),
flash-style causal attention for its 4 heads, and a partial output
projection over its 256 head-dims (row-parallel).  The host sums the 4
partial projections per batch and adds bproj.

v2 layout/engine plan (vs the fp32r baseline):
  - all matmul inputs bf16 (x / Wqkv / Wproj cast on host); psum fp32.
  - score matmuls row-tiled per head: lhsT/rhs are 64-partition slices of
    kz/qT (head h in rows 64h:64h+64), so the two heads of a pair run
    CONCURRENTLY on different PE row-groups (auto tile_position).
  - per k-step the two heads' score tiles land in one [128,1024] 2-bank
    psum tile; ONE ACT exp covers both heads (halves ACT instr count).
    ACT does nothing else; psum->sbuf copies run on DVE.
  - causal diag masking by multiplying pt with a 0/1 bf16 triangle on
    gpsimd (pt is SBUF; gpsimd has no PSUM port) - off ACT and psum path.
  - softmax 1/Z: DVE reciprocal_approx_fast on the AV ones-column rows
    [1,1024], gpsimd partition_broadcast to [64,1024], DVE multiplies the
    unnormalized AV psum rows directly (h1 via SBUF tmp + DMA row shift).
  - QKV biases folded into the matmul accumulation as K=1 rank-1 mms
    (bias x ones-row); output projection bias added on host.
  - projection psums DMA'd straight from PSUM to DRAM (no engine copy).
  - emission order s-outer: QKV(s) ... attn(pair0,s) attn(pair1,s) with
    proj(s-1) interleaved, so PE never drains between phases.
"""

import sys
import types

import numpy as np


def _install_ntff_shim():
    """The container's antenv stub lacks axon_hooks; recreate it so
    run_bass_kernel_spmd(trace=True) can reach the NTFF profiler."""
    if "antenv.axon_hooks" in sys.modules:
        return
    try:
        import antenv
        from trn_agent_boot.trn_boot import _ntff_profile_via_ctypes
    except Exception:
        return
    mod = types.ModuleType("antenv.axon_hooks")
    hook = _ntff_profile_via_ctypes("/opt/axon/libaxon_pjrt.so")
    mod.get_axon_ntff_profile_hook = lambda: hook
    mod.set_axon_ntff_profile_hook = lambda h: None
    sys.modules["antenv.axon_hooks"] = mod
    antenv.axon_hooks = mod


_install_ntff_shim()

import ml_dtypes  # noqa: E402

import concourse.bass as bass  # noqa: E402
import concourse.mybir as mybir  # noqa: E402
import concourse.tile as tile  # noqa: E402
from concourse.bass_utils import run_bass_kernel_spmd  # noqa: E402
from concourse.vector_clock import ScopedClock, VectorClock  # noqa: E402

B, L, D, H = 2, 2048, 1024, 16
HD = D // H  # 64
N_CORES = 8
HPC = 4  # heads per core
CD = HPC * HD  # 256 head-dims per core
VW = HPC * (HD + 1)  # 260 interleaved V columns (64 vals + ones col per head)
SCALE = HD**-0.5  # 0.125
F32 = mybir.dt.float32
R32 = mybir.dt.float32r  # same bits as fp32; full-rate matmul operand
B16 = mybir.dt.bfloat16
NPB16 = ml_dtypes.bfloat16

KT = L // 128  # 16 k-tiles of 128 keys
NS = L // 512  # 4 query chunks of 512
N_DK = D // 128  # 8 feature k-tiles
AV_DELAY = 6  # AV matmul issues this many (k,h)-steps behind its exp


class _TileContext(tile.TileContext):
    """Split exit-drain sem waits to 1 per drain; this walrus build's
    CTRL codegen rejects drains with 2+ sync waits."""

    def _drain_and_barrier(self, tick_clock, wait_clock):
        g = tick_clock.global_clock
        n = len(g)
        procs = [i for i in range(n) if g[i] > 0]
        for p in procs:
            vec = [g[i] if i == p else 0 for i in range(n)]
            d = self.nc.sync.drain()
            wait_clock.add_sem_waits(d.ins, ScopedClock({None: VectorClock(vec)}))
        self.nc.all_engine_barrier()
        popped = self.nc._tile_sem_poison_stack.pop()
        assert popped is self._sem_poison
        self.nc.clear_and_free_semaphores(list(self.sems.allocated().values()))
        self.nc.all_engine_barrier()


def _split_multi_waits(nc):
    """This walrus build's codegen accepts only ONE sync wait per
    instruction; hoist extra waits onto preceding same-engine NOPs."""
    for f in nc.m.functions:
        for blk in f.blocks:
            orig = list(blk.instructions)
            expanded = []
            changed = False
            for ins in orig:
                si = ins.sync_info
                if si is not None and si.on_wait is not None and len(si.on_wait) > 1:
                    changed = True
                    waits = list(si.on_wait)
                    eng = nc.engines[ins.engine]
                    for w in waits[:-1]:
                        nop = eng.nop(nofuse=True).ins
                        # eng.nop() auto-appends to the CURRENT bb; pull it
                        # out -- we re-insert it before `ins` in ins's bb.
                        nc.cur_bb.bb.instructions.remove(nop)
                        nop.sync_info = mybir.SyncInfo(on_wait=[w], on_update=[])
                        expanded.append(nop)
                    ins.sync_info = mybir.SyncInfo(
                        on_wait=[waits[-1]], on_update=list(si.on_update or [])
                    )
                expanded.append(ins)
            if changed:
                il = blk.instructions
                for ins in list(il):
                    il.remove(ins)
                for ins in expanded:
                    il.append(ins)


def _build_program():
    nc = bass.Bass()
    # host supplies x / wqkv / wproj PRE-BLOCKED into SBUF layout
    # ([128 partitions, grouped cols]) so every staging DMA is one plain 2D
    # contiguous transfer with 2-8KB per-partition lines (thin lines ran at
    # ~280GB/s and gated the start of compute)
    xT_d = nc.dram_tensor("xT", [128, N_DK * L], B16, kind="ExternalInput").ap()
    wqkv_d = nc.dram_tensor("wqkv", [128, N_DK * (2 * CD + VW)], B16, kind="ExternalInput").ap()
    bqk_d = nc.dram_tensor("bqk", [1, 2 * CD], B16, kind="ExternalInput").ap()
    bv_d = nc.dram_tensor("bv", [1, VW], B16, kind="ExternalInput").ap()
    wproj_d = nc.dram_tensor("wproj", [128, 2 * D], B16, kind="ExternalInput").ap()
    ones_d = nc.dram_tensor("onesr", [1, 512], B16, kind="ExternalInput").ap()
    ones64_d = nc.dram_tensor("ones64", [1, 64], R32, kind="ExternalInput").ap()
    tri_d = nc.dram_tensor("tri01", [128, 128], B16, kind="ExternalInput").ap()
    yT_d = nc.dram_tensor("yT", [D, L], B16, kind="ExternalOutput").ap()

    mm = nc.tensor.matmul
    MUL = mybir.AluOpType.mult
    EXP = mybir.ActivationFunctionType.Exp

    with _TileContext(nc) as tc, tc.tile_pool(name="sb", bufs=1) as sb, tc.tile_pool(
        name="ps", bufs=1, space="PSUM"
    ) as ps:
        # ---- persistent SBUF tensors / staged input DMA ----
        # host-blocked col layouts:
        #   wq_all: [qm0 | qm1 | km0 | km1 | v], m-tile block = 128*8k cols
        #   xT_all: chunk-major: col = 4096*s + 512*k + t
        WQW = 2 * CD + VW  # 772
        wq_all = sb.tile([128, N_DK * WQW], B16, tag="wq", bufs=1, name="wq")
        xT_all = sb.tile([128, N_DK * L], B16, tag="xT", bufs=1, name="xT")
        wproj_all = sb.tile([128, 2 * D], B16, tag="wproj", bufs=1, name="wproj")

        def wq_qk(k, m, c0, c1):
            base = 1024 * m + 128 * k
            return wq_all[:, base + c0 : base + c1]

        def wq_v(k):
            return wq_all[:, 4096 + VW * k : 4096 + VW * (k + 1)]

        def xtc(k, s, j0=0, j1=512):
            base = 4096 * s + 512 * k
            return xT_all[:, base + j0 : base + j1]

        # staging order = first-need order: the very first psum tile only
        # needs the qm0 block + x chunk0 k<4 before its first matmuls
        nc.sync.dma_start(out=wq_all[:, 0:1024], in_=wqkv_d[:, 0:1024])
        nc.sync.dma_start(out=xT_all[:, 0:2048], in_=xT_d[:, 0:2048])
        nc.sync.dma_start(out=xT_all[:, 2048:4096], in_=xT_d[:, 2048:4096])
        nc.sync.dma_start(out=wq_all[:, 1024:2048], in_=wqkv_d[:, 1024:2048])
        ones = sb.tile([1, 512], B16, tag="ones", bufs=1)
        nc.sync.dma_start(out=ones[:], in_=ones_d[:])
        ones64 = sb.tile([1, 64], R32, tag="ones64", bufs=1)
        nc.sync.dma_start(out=ones64[:], in_=ones64_d[:])
        tri = sb.tile([128, 128], B16, tag="tri", bufs=1)
        nc.sync.dma_start(out=tri[:], in_=tri_d[:])
        bqk = sb.tile([1, 2 * CD], B16, tag="bqk", bufs=1)
        nc.sync.dma_start(out=bqk[:], in_=bqk_d[:])
        bv = sb.tile([1, VW], B16, tag="bv", bufs=1)
        nc.sync.dma_start(out=bv[:], in_=bv_d[:])
        nc.sync.dma_start(out=wq_all[:, 2048:4096], in_=wqkv_d[:, 2048:4096])
        nc.sync.dma_start(out=wq_all[:, 4096:6176], in_=wqkv_d[:, 4096:6176])
        for s in range(1, NS):
            nc.sync.dma_start(
                out=xT_all[:, 4096 * s : 4096 * (s + 1)],
                in_=xT_d[:, 4096 * s : 4096 * (s + 1)],
            )
        nc.sync.dma_start(out=wproj_all[:], in_=wproj_d[:])

        def wproj_c(kt, c0, c1):
            return wproj_all[:, D * kt + c0 : D * kt + c1]

        # Q^T / K^T packed per pair: rows 0-63 = head 2p, 64-127 = head 2p+1
        qT = [sb.tile([128, L], B16, tag=f"qT{p}", bufs=1, name=f"qT{p}") for p in range(2)]
        kz = [sb.tile([128, L], B16, tag=f"kz{p}", bufs=1, name=f"kz{p}") for p in range(2)]
        # V natural layout, 16 token tiles of [128, 4*65]; col 64 of each
        # head group = 1.0 (zero weight cols + the bv ones-row rank-1 mm)
        vsb = [sb.tile([128, VW], B16, tag=f"v{t}", bufs=1, name=f"v{t}") for t in range(KT)]
        attnT = [
            sb.tile([128, L], B16, tag=f"attnT{k}", bufs=1, name=f"attnT{k}")
            for k in range(2)
        ]

        def emit_qkv_mp(s, mp):
            # Q/K part: psum pair tile, halves m=2mp and 2mp+1 -> [wcol, tok]
            cs = slice(512 * s, 512 * (s + 1))
            dstA, dstB = [(qT[0], qT[1]), (kz[0], kz[1])][mp]
            p = ps.tile([128, 1024], F32, tag="ps", bufs=2, name="qkps")
            for mi in range(2):
                m = 2 * mp + mi
                half = slice(512 * mi, 512 * mi + 512)
                for k in range(N_DK):
                    mm(
                        p[:, half],
                        wq_qk(k, m, 0, 128),
                        xtc(k, s),
                        start=(k == 0),
                        stop=False,
                    )
                mm(
                    p[:, half],
                    bqk[0:1, 128 * m : 128 * (m + 1)],
                    ones[:],
                    start=False,
                    stop=True,
                )
            nc.vector.tensor_copy(dstA[:, cs], p[:, 0:512])
            nc.vector.tensor_copy(dstB[:, cs], p[:, 512:1024])

        def emit_qkv_vp(s, jp):
            # V part: halves j=2jp and 2jp+1 -> [token, vcol]
            p = ps.tile([128, 1024], F32, tag="ps", bufs=2, name="vps")
            for ji in range(2):
                j = 2 * jp + ji
                off = 512 * ji
                for k in range(N_DK):
                    mm(
                        p[:, off : off + VW],
                        xtc(k, s, 128 * j, 128 * (j + 1)),
                        wq_v(k),
                        start=(k == 0),
                        stop=False,
                    )
                mm(
                    p[:, off : off + VW],
                    ones[0:1, 0:128],
                    bv[:],
                    start=False,
                    stop=True,
                )
            nc.vector.tensor_copy(vsb[4 * s + 2 * jp][:], p[:, 0:VW])
            nc.vector.tensor_copy(vsb[4 * s + 2 * jp + 1][:], p[:, 512 : 512 + VW])

        def emit_proj_mp(s, mp):
            cs = slice(512 * s, 512 * (s + 1))
            p = ps.tile([128, 1024], F32, tag="ps", bufs=2, name="projps")
            for mi in range(2):
                m = 2 * mp + mi
                half = slice(512 * mi, 512 * mi + 512)
                for kt in range(2):
                    mm(
                        p[:, half],
                        wproj_c(kt, 128 * m, 128 * (m + 1)),
                        attnT[kt][:, cs],
                        start=(kt == 0),
                        stop=(kt == 1),
                    )
            ysb = sb.tile([128, 1024], B16, tag="ysb", bufs=3, name="ysb")
            # alternate the psum pull between ACT (copy shares exp's table
            # set, no reload) and DVE to keep either from being the hotspot
            if mp % 2 == 0:
                nc.scalar.copy(ysb[:], p[:])
            else:
                nc.vector.tensor_copy(ysb[:], p[:])
            nc.sync.dma_start(
                out=yT_d[256 * mp : 256 * (mp + 1), cs].rearrange(
                    "(m p) c -> p m c", m=2
                ),
                in_=ysb[:].rearrange("p (m c) -> p m c", m=2),
            )

        # ================= attention =================
        # Software-pipelined: AV mms trail their exp by AV_DELAY (k,h)-steps.
        # Block b-1's tail AVs flush + its 1/Z chain start early in block b;
        # the broadcast + normalize land a few k-steps later, and deferred
        # QKV/proj tiles are popped one per k-step to fill PE slack.
        def emit_norm_a(bid):
            # 1/Z chain, all off ACT: copy Z row out of psum, DMA-reshape to
            # [128,8] so the (slow, precise) DVE reciprocal runs 8 elems per
            # lane, DMA-reshape back.  un-copy frees the av psum banks.
            pair, s, av = blocks_done[bid]
            z_sb = sb.tile([1, 1024], F32, tag="z", bufs=2, name="z")
            nc.vector.tensor_copy(z_sb[:], av[64:65, :])
            zt = sb.tile([128, 8], F32, tag="zt", bufs=2, name="zt")
            nc.sync.dma_start(out=zt[:], in_=z_sb[:])
            un = sb.tile([64, 1024], B16, tag="un", bufs=2, name="un")
            nc.vector.tensor_copy(un[:], av[0:64, :])
            # write the reciprocal as f32r (same bits as f32) so the bc
            # matmul can read zr directly -- a bitcast AP here would break
            # the tile framework's dependency tracking (different tensor obj)
            ztr = sb.tile([128, 8], R32, tag="ztr", bufs=2, name="ztr")
            with nc.allow_low_precision(reason="softmax 1/Z"):
                nc.vector.reciprocal(ztr[:], zt[:])
            zr = sb.tile([1, 1024], R32, tag="zr", bufs=2, name="zr")
            nc.sync.dma_start(out=zr[:], in_=ztr[:])
            return (pair, s, un, zr)

        def emit_norm_b(norm_st):
            pair, s, un, zr = norm_st
            q0 = 512 * s
            # broadcast 1/Z to 64 partitions via K=1 rank-1 matmuls
            # (fp32 moving operand max is 512 wide -> one mm per half)
            bc = ps.tile([64, 1024], F32, tag="ps", bufs=2, name="bcps")
            mm(bc[:, 0:512], ones64[:], zr[:, 0:512], start=True, stop=True)
            mm(bc[:, 512:1024], ones64[:], zr[:, 512:1024], start=True, stop=True)
            nc.vector.tensor_tensor(
                attnT[pair][0:64, q0 : q0 + 512], un[:, 0:512], bc[:, 0:512], op=MUL
            )
            ntmp = sb.tile([64, 512], B16, tag="ntmp", bufs=2, name="ntmp")
            nc.vector.tensor_tensor(ntmp[:], un[:, 512:1024], bc[:, 512:1024], op=MUL)
            nc.sync.dma_start(out=attnT[pair][64:128, q0 : q0 + 512], in_=ntmp[:])

        pending = []  # (block_id, mm_args, mm_kwargs)
        blocks_done = {}  # bid -> (pair, s, av)
        fin_prev = None  # bid awaiting tail-flush + norm
        norm_st = None  # phase-A output awaiting broadcast+normalize
        deferred = []  # QKV/proj tile thunks, popped one per k-step
        blocks = [(p, s) for s in range(NS) for p in range(2)]
        for bid, (pair, s) in enumerate(blocks):
            if bid == 0:
                for mp in range(2):
                    emit_qkv_mp(0, mp)
                for jp in range(2):
                    emit_qkv_vp(0, jp)
            if pair == 0 and s + 1 < NS:
                deferred += [
                    lambda s=s, mp=mp: emit_qkv_mp(s + 1, mp) for mp in range(2)
                ]
                deferred += [
                    lambda s=s, jp=jp: emit_qkv_vp(s + 1, jp) for jp in range(2)
                ]
            q0 = 512 * s
            n_k = 4 * s + 4
            av = ps.tile([65, 1024], F32, tag="av", bufs=2, name="av")
            for k in range(n_k):
                k0 = 128 * k
                diag_t = k - 4 * s
                lo = 128 * diag_t if diag_t >= 0 else 0
                sp = ps.tile([128, 1024], F32, tag="ps", bufs=2)
                for h in range(2):
                    hb = slice(64 * h, 64 * h + 64)
                    off = 512 * h
                    mm(
                        sp[:, off + lo : off + 512],
                        kz[pair][hb, k0 : k0 + 128],
                        qT[pair][hb, q0 + lo : q0 + 512],
                        start=True,
                        stop=True,
                    )
                pt = sb.tile([128, 1024], B16, tag="pt", bufs=AV_DELAY + 2)
                if diag_t >= 0:
                    for h in range(2):
                        off = 512 * h
                        nc.scalar.activation(
                            pt[:, off + lo : off + 512],
                            sp[:, off + lo : off + 512],
                            EXP,
                            scale=SCALE,
                        )
                        nc.gpsimd.tensor_tensor(
                            pt[:, off + lo : off + lo + 128],
                            pt[:, off + lo : off + lo + 128],
                            tri[:],
                            op=MUL,
                        )
                else:
                    nc.scalar.activation(pt[:], sp[:], EXP, scale=SCALE)
                for h in range(2):
                    hg = 2 * pair + h
                    off = 512 * h
                    pending.append(
                        (
                            bid,
                            (
                                av[0:65, off + lo : off + 512],
                                vsb[k][:, 65 * hg : 65 * hg + 65],
                                pt[:, off + lo : off + 512],
                            ),
                            dict(
                                start=(k == 0),
                                stop=(k == n_k - 1),
                                skip_group_check=True,
                            ),
                        )
                    )
                    while len(pending) > AV_DELAY:
                        _, a, kw = pending.pop(0)
                        mm(*a, **kw)
                if k == 1 and fin_prev is not None:
                    # norm_b of the block BEFORE last: its 1/Z reshape chain
                    # (DVE + 2 small DMAs, ~5us latency) was started a full
                    # block ago, so its bc matmul no longer stalls the PE
                    if norm_st is not None:
                        npair, nspr = norm_st[0], norm_st[1]
                        emit_norm_b(norm_st)
                        norm_st = None
                        # proj(s) only after BOTH pairs' chunk-s normalizes
                        # are emitted -- popping any earlier would read attnT
                        # rows that have no producer yet
                        if npair == 1:
                            deferred += [
                                lambda s=nspr, mp=mp: emit_proj_mp(s, mp)
                                for mp in range(4)
                            ]
                    # independent PE work BEFORE the tail flush: the flushed
                    # AVs wait on the previous block's last exps (ACT is
                    # still catching up); these tiles keep the PE warm
                    for _ in range(2):
                        if deferred:
                            deferred.pop(0)()
                    pbid = fin_prev
                    while pending and pending[0][0] == pbid:
                        _, a, kw = pending.pop(0)
                        mm(*a, **kw)
                    norm_st = emit_norm_a(pbid)
                    fin_prev = None
                elif k >= 2 and deferred:
                    deferred.pop(0)()
            blocks_done[bid] = (pair, s, av)
            fin_prev = bid
        while pending:
            _, a, kw = pending.pop(0)
            mm(*a, **kw)
        last_st = emit_norm_a(fin_prev)
        if norm_st is not None:
            emit_norm_b(norm_st)
        while deferred:
            deferred.pop(0)()
        emit_norm_b(last_st)
        for mp in range(4):
            emit_proj_mp(NS - 1, mp)
    _split_multi_waits(nc)
    return nc


_NC_CACHE = None
LAST_RESULTS = None

_ONES = np.ones((1, 512), dtype=NPB16)
_ONES64 = np.ones((1, 64), dtype=np.float32)
_I, _J = np.meshgrid(np.arange(128), np.arange(128), indexing="ij")
_TRI01 = np.where(_J >= _I, 1.0, 0.0).astype(NPB16)


def _make_in_maps(x, Wqkv, bqkv, Wproj, bproj):
    in_maps = []
    for c in range(N_CORES):
        b, g = divmod(c, 4)
        qc = slice(CD * g, CD * (g + 1))
        wqc = Wqkv[:, qc]
        wk = Wqkv[:, D : 2 * D][:, qc]
        wv = Wqkv[:, 2 * D : 3 * D][:, qc]
        bq = bqkv[qc]
        bk = bqkv[D : 2 * D][qc]
        bvv = bqkv[2 * D : 3 * D][qc]
        # V columns interleaved per head: [wv_h (64 cols) | zero col]; the
        # ones-row rank-1 matmul adds [bv_h | 1.0] so Z rides along in AV.
        wv_i = np.zeros((D, VW), dtype=np.float32)
        bv_i = np.zeros((1, VW), dtype=np.float32)
        for h in range(HPC):
            wv_i[:, 65 * h : 65 * h + 64] = wv[:, 64 * h : 64 * h + 64]
            bv_i[0, 65 * h : 65 * h + 64] = bvv[64 * h : 64 * h + 64]
            bv_i[0, 65 * h + 64] = 1.0
        # host-blocked SBUF layouts (partition-major, fat contiguous lines)
        def kblock(a, width):
            # [1024, width] -> [128, 8*width] with col = width*k + c
            return a.reshape(N_DK, 128, width).transpose(1, 0, 2).reshape(128, -1)

        xT = x[b].T  # [D, L]
        xT_blk = (
            xT.reshape(N_DK, 128, NS, 512)
            .transpose(1, 2, 0, 3)
            .reshape(128, N_DK * L)
        )  # col = 4096*s + 512*k + t
        wq_blk = np.concatenate(
            [
                kblock(np.ascontiguousarray(wqc[:, 0:128]), 128),
                kblock(np.ascontiguousarray(wqc[:, 128:256]), 128),
                kblock(np.ascontiguousarray(wk[:, 0:128]), 128),
                kblock(np.ascontiguousarray(wk[:, 128:256]), 128),
                kblock(wv_i, VW),
            ],
            axis=1,
        )
        wproj_blk = (
            Wproj[qc, :].reshape(2, 128, D).transpose(1, 0, 2).reshape(128, 2 * D)
        )
        in_maps.append(
            {
                "xT": np.ascontiguousarray(xT_blk).astype(NPB16),
                "wqkv": np.ascontiguousarray(wq_blk).astype(NPB16),
                "bqk": np.concatenate([bq, bk]).reshape(1, 2 * CD).astype(NPB16),
                "bv": bv_i.astype(NPB16),
                "wproj": np.ascontiguousarray(wproj_blk).astype(NPB16),
                "onesr": _ONES,
                "ones64": _ONES64,
                "tri01": _TRI01,
            }
        )

    return in_maps


def kernel(x, Wqkv, bqkv, Wproj, bproj):
    global _NC_CACHE, LAST_RESULTS
    x = np.asarray(x, dtype=np.float32)
    Wqkv = np.asarray(Wqkv, dtype=np.float32)
    bqkv = np.asarray(bqkv, dtype=np.float32)
    Wproj = np.asarray(Wproj, dtype=np.float32)
    bproj = np.asarray(bproj, dtype=np.float32)

    if _NC_CACHE is None:
        _NC_CACHE = _build_program()
    nc = _NC_CACHE

    in_maps = _make_in_maps(x, Wqkv, bqkv, Wproj, bproj)
    res = run_bass_kernel_spmd(nc, in_maps, core_ids=list(range(N_CORES)))
    LAST_RESULTS = res

    out = np.empty((B, L, D), dtype=np.float32)
    for b in range(B):
        acc = res.results[4 * b]["yT"].astype(np.float32)
        for g in range(1, 4):
            acc = acc + res.results[4 * b + g]["yT"]
        out[b] = acc.T + bproj[None, :]
    return out


# revision 33
# speedup vs baseline: 1.0059x; 1.0059x over previous
"""Causal self-attention (B=2, L=2048, D=1024, H=16) on 8 trn2 NeuronCores.

Sharding: core c = 4*b + g handles batch b and head group g (4 heads).
Per core: QKV projection for its heads' weight columns (tensor-parallel),
flash-style causal attention for its 4 heads, and a partial output
projection over its 256 head-dims (row-parallel).  The host sums the 4
partial projections per batch and adds bproj.

v2 layout/engine plan (vs the fp32r baseline):
  - all matmul inputs bf16 (x / Wqkv / Wproj cast on host); psum fp32.
  - score matmuls row-tiled per head: lhsT/rhs are 64-partition slices of
    kz/qT (head h in rows 64h:64h+64), so the two heads of a pair run
    CONCURRENTLY on different PE row-groups (auto tile_position).
  - per k-step the two heads' score tiles land in one [128,1024] 2-bank
    psum tile; ONE ACT exp covers both heads (halves ACT instr count).
    ACT does nothing else; psum->sbuf copies run on DVE.
  - causal diag masking by multiplying pt with a 0/1 bf16 triangle on
    gpsimd (pt is SBUF; gpsimd has no PSUM port) - off ACT and psum path.
  - softmax 1/Z: DVE reciprocal_approx_fast on the AV ones-column rows
    [1,1024], gpsimd partition_broadcast to [64,1024], DVE multiplies the
    unnormalized AV psum rows directly (h1 via SBUF tmp + DMA row shift).
  - QKV biases folded into the matmul accumulation as K=1 rank-1 mms
    (bias x ones-row); output projection bias added on host.
  - projection psums DMA'd straight from PSUM to DRAM (no engine copy).
  - emission order s-outer: QKV(s) ... attn(pair0,s) attn(pair1,s) with
    proj(s-1) interleaved, so PE never drains between phases.
"""

import sys
import types

import numpy as np


def _install_ntff_shim():
    """The container's antenv stub lacks axon_hooks; recreate it so
    run_bass_kernel_spmd(trace=True) can reach the NTFF profiler."""
    if "antenv.axon_hooks" in sys.modules:
        return
    try:
        import antenv
        from trn_agent_boot.trn_boot import _ntff_profile_via_ctypes
    except Exception:
        return
    mod = types.ModuleType("antenv.axon_hooks")
    hook = _ntff_profile_via_ctypes("/opt/axon/libaxon_pjrt.so")
    mod.get_axon_ntff_profile_hook = lambda: hook
    mod.set_axon_ntff_profile_hook = lambda h: None
    sys.modules["antenv.axon_hooks"] = mod
    antenv.axon_hooks = mod


_install_ntff_shim()

import ml_dtypes  # noqa: E402

import concourse.bass as bass  # noqa: E402
import concourse.mybir as mybir  # noqa: E402
import concourse.tile as tile  # noqa: E402
from concourse.bass_utils import run_bass_kernel_spmd  # noqa: E402
from concourse.vector_clock import ScopedClock, VectorClock  # noqa: E402

B, L, D, H = 2, 2048, 1024, 16
HD = D // H  # 64
N_CORES = 8
HPC = 4  # heads per core
CD = HPC * HD  # 256 head-dims per core
VW = HPC * (HD + 1)  # 260 interleaved V columns (64 vals + ones col per head)
SCALE = HD**-0.5  # 0.125
F32 = mybir.dt.float32
R32 = mybir.dt.float32r  # same bits as fp32; full-rate matmul operand
B16 = mybir.dt.bfloat16
NPB16 = ml_dtypes.bfloat16

KT = L // 128  # 16 k-tiles of 128 keys
NS = L // 512  # 4 query chunks of 512
N_DK = D // 128  # 8 feature k-tiles
AV_DELAY = 4  # AV matmul issues this many (k,h)-steps behind its exp


class _TileContext(tile.TileContext):
    """Split exit-drain sem waits to 1 per drain; this walrus build's
    CTRL codegen rejects drains with 2+ sync waits."""

    def _drain_and_barrier(self, tick_clock, wait_clock):
        g = tick_clock.global_clock
        n = len(g)
        procs = [i for i in range(n) if g[i] > 0]
        for p in procs:
            vec = [g[i] if i == p else 0 for i in range(n)]
            d = self.nc.sync.drain()
            wait_clock.add_sem_waits(d.ins, ScopedClock({None: VectorClock(vec)}))
        self.nc.all_engine_barrier()
        popped = self.nc._tile_sem_poison_stack.pop()
        assert popped is self._sem_poison
        self.nc.clear_and_free_semaphores(list(self.sems.allocated().values()))
        self.nc.all_engine_barrier()


def _split_multi_waits(nc):
    """This walrus build's codegen accepts only ONE sync wait per
    instruction; hoist extra waits onto preceding same-engine NOPs."""
    for f in nc.m.functions:
        for blk in f.blocks:
            orig = list(blk.instructions)
            expanded = []
            changed = False
            for ins in orig:
                si = ins.sync_info
                if si is not None and si.on_wait is not None and len(si.on_wait) > 1:
                    changed = True
                    waits = list(si.on_wait)
                    eng = nc.engines[ins.engine]
                    for w in waits[:-1]:
                        nop = eng.nop(nofuse=True).ins
                        # eng.nop() auto-appends to the CURRENT bb; pull it
                        # out -- we re-insert it before `ins` in ins's bb.
                        nc.cur_bb.bb.instructions.remove(nop)
                        nop.sync_info = mybir.SyncInfo(on_wait=[w], on_update=[])
                        expanded.append(nop)
                    ins.sync_info = mybir.SyncInfo(
                        on_wait=[waits[-1]], on_update=list(si.on_update or [])
                    )
                expanded.append(ins)
            if changed:
                il = blk.instructions
                for ins in list(il):
                    il.remove(ins)
                for ins in expanded:
                    il.append(ins)


def _build_program():
    nc = bass.Bass()
    # host supplies x / wqkv / wproj PRE-BLOCKED into SBUF layout
    # ([128 partitions, grouped cols]) so every staging DMA is one plain 2D
    # contiguous transfer with 2-8KB per-partition lines (thin lines ran at
    # ~280GB/s and gated the start of compute)
    xT_d = nc.dram_tensor("xT", [128, N_DK * L], B16, kind="ExternalInput").ap()
    wqkv_d = nc.dram_tensor("wqkv", [128, N_DK * (2 * CD + VW)], B16, kind="ExternalInput").ap()
    bqk_d = nc.dram_tensor("bqk", [1, 2 * CD], B16, kind="ExternalInput").ap()
    bv_d = nc.dram_tensor("bv", [1, VW], B16, kind="ExternalInput").ap()
    wproj_d = nc.dram_tensor("wproj", [128, 2 * D], B16, kind="ExternalInput").ap()
    ones_d = nc.dram_tensor("onesr", [1, 512], B16, kind="ExternalInput").ap()
    ones64_d = nc.dram_tensor("ones64", [1, 64], R32, kind="ExternalInput").ap()
    tri_d = nc.dram_tensor("tri01", [128, 128], B16, kind="ExternalInput").ap()
    yT_d = nc.dram_tensor("yT", [D, L], B16, kind="ExternalOutput").ap()

    mm = nc.tensor.matmul
    MUL = mybir.AluOpType.mult
    EXP = mybir.ActivationFunctionType.Exp

    with _TileContext(nc) as tc, tc.tile_pool(name="sb", bufs=1) as sb, tc.tile_pool(
        name="ps", bufs=1, space="PSUM"
    ) as ps:
        # ---- persistent SBUF tensors / staged input DMA ----
        # host-blocked col layouts:
        #   wq_all: [qm0 | qm1 | km0 | km1 | v], m-tile block = 128*8k cols
        #   xT_all: chunk-major: col = 4096*s + 512*k + t
        WQW = 2 * CD + VW  # 772
        wq_all = sb.tile([128, N_DK * WQW], B16, tag="wq", bufs=1, name="wq")
        xT_all = sb.tile([128, N_DK * L], B16, tag="xT", bufs=1, name="xT")
        wproj_all = sb.tile([128, 2 * D], B16, tag="wproj", bufs=1, name="wproj")

        def wq_qk(k, m, c0, c1):
            base = 1024 * m + 128 * k
            return wq_all[:, base + c0 : base + c1]

        def wq_v(k):
            return wq_all[:, 4096 + VW * k : 4096 + VW * (k + 1)]

        def xtc(k, s, j0=0, j1=512):
            base = 4096 * s + 512 * k
            return xT_all[:, base + j0 : base + j1]

        # staging order = first-need order: the very first psum tile only
        # needs the qm0 block + x chunk0 k<4 before its first matmuls
        nc.sync.dma_start(out=wq_all[:, 0:1024], in_=wqkv_d[:, 0:1024])
        nc.sync.dma_start(out=xT_all[:, 0:2048], in_=xT_d[:, 0:2048])
        nc.sync.dma_start(out=xT_all[:, 2048:4096], in_=xT_d[:, 2048:4096])
        nc.sync.dma_start(out=wq_all[:, 1024:2048], in_=wqkv_d[:, 1024:2048])
        ones = sb.tile([1, 512], B16, tag="ones", bufs=1)
        nc.sync.dma_start(out=ones[:], in_=ones_d[:])
        ones64 = sb.tile([1, 64], R32, tag="ones64", bufs=1)
        nc.sync.dma_start(out=ones64[:], in_=ones64_d[:])
        tri = sb.tile([128, 128], B16, tag="tri", bufs=1)
        nc.sync.dma_start(out=tri[:], in_=tri_d[:])
        bqk = sb.tile([1, 2 * CD], B16, tag="bqk", bufs=1)
        nc.sync.dma_start(out=bqk[:], in_=bqk_d[:])
        bv = sb.tile([1, VW], B16, tag="bv", bufs=1)
        nc.sync.dma_start(out=bv[:], in_=bv_d[:])
        nc.sync.dma_start(out=wq_all[:, 2048:4096], in_=wqkv_d[:, 2048:4096])
        nc.sync.dma_start(out=wq_all[:, 4096:6176], in_=wqkv_d[:, 4096:6176])
        for s in range(1, NS):
            nc.sync.dma_start(
                out=xT_all[:, 4096 * s : 4096 * (s + 1)],
                in_=xT_d[:, 4096 * s : 4096 * (s + 1)],
            )
        nc.sync.dma_start(out=wproj_all[:], in_=wproj_d[:])

        def wproj_c(kt, c0, c1):
            return wproj_all[:, D * kt + c0 : D * kt + c1]

        # Q^T / K^T packed per pair: rows 0-63 = head 2p, 64-127 = head 2p+1
        qT = [sb.tile([128, L], B16, tag=f"qT{p}", bufs=1, name=f"qT{p}") for p in range(2)]
        kz = [sb.tile([128, L], B16, tag=f"kz{p}", bufs=1, name=f"kz{p}") for p in range(2)]
        # V natural layout, 16 token tiles of [128, 4*65]; col 64 of each
        # head group = 1.0 (zero weight cols + the bv ones-row rank-1 mm)
        vsb = [sb.tile([128, VW], B16, tag=f"v{t}", bufs=1, name=f"v{t}") for t in range(KT)]
        attnT = [
            sb.tile([128, L], B16, tag=f"attnT{k}", bufs=1, name=f"attnT{k}")
            for k in range(2)
        ]

        def emit_qkv_mp(s, mp):
            # Q/K part: psum pair tile, halves m=2mp and 2mp+1 -> [wcol, tok]
            cs = slice(512 * s, 512 * (s + 1))
            dstA, dstB = [(qT[0], qT[1]), (kz[0], kz[1])][mp]
            p = ps.tile([128, 1024], F32, tag="ps", bufs=2, name="qkps")
            for mi in range(2):
                m = 2 * mp + mi
                half = slice(512 * mi, 512 * mi + 512)
                for k in range(N_DK):
                    mm(
                        p[:, half],
                        wq_qk(k, m, 0, 128),
                        xtc(k, s),
                        start=(k == 0),
                        stop=False,
                    )
                mm(
                    p[:, half],
                    bqk[0:1, 128 * m : 128 * (m + 1)],
                    ones[:],
                    start=False,
                    stop=True,
                )
            nc.vector.tensor_copy(dstA[:, cs], p[:, 0:512])
            nc.vector.tensor_copy(dstB[:, cs], p[:, 512:1024])

        def emit_qkv_vp(s, jp):
            # V part: halves j=2jp and 2jp+1 -> [token, vcol]
            p = ps.tile([128, 1024], F32, tag="ps", bufs=2, name="vps")
            for ji in range(2):
                j = 2 * jp + ji
                off = 512 * ji
                for k in range(N_DK):
                    mm(
                        p[:, off : off + VW],
                        xtc(k, s, 128 * j, 128 * (j + 1)),
                        wq_v(k),
                        start=(k == 0),
                        stop=False,
                    )
                mm(
                    p[:, off : off + VW],
                    ones[0:1, 0:128],
                    bv[:],
                    start=False,
                    stop=True,
                )
            nc.vector.tensor_copy(vsb[4 * s + 2 * jp][:], p[:, 0:VW])
            nc.vector.tensor_copy(vsb[4 * s + 2 * jp + 1][:], p[:, 512 : 512 + VW])

        def emit_proj_mp(s, mp):
            cs = slice(512 * s, 512 * (s + 1))
            p = ps.tile([128, 1024], F32, tag="ps", bufs=2, name="projps")
            for mi in range(2):
                m = 2 * mp + mi
                half = slice(512 * mi, 512 * mi + 512)
                for kt in range(2):
                    mm(
                        p[:, half],
                        wproj_c(kt, 128 * m, 128 * (m + 1)),
                        attnT[kt][:, cs],
                        start=(kt == 0),
                        stop=(kt == 1),
                    )
            ysb = sb.tile([128, 1024], B16, tag="ysb", bufs=3, name="ysb")
            # alternate the psum pull between ACT (copy shares exp's table
            # set, no reload) and DVE to keep either from being the hotspot
            if mp % 2 == 0:
                nc.scalar.copy(ysb[:], p[:])
            else:
                nc.vector.tensor_copy(ysb[:], p[:])
            nc.sync.dma_start(
                out=yT_d[256 * mp : 256 * (mp + 1), cs].rearrange(
                    "(m p) c -> p m c", m=2
                ),
                in_=ysb[:].rearrange("p (m c) -> p m c", m=2),
            )

        # ================= attention =================
        # Software-pipelined: AV mms trail their exp by AV_DELAY (k,h)-steps.
        # Block b-1's tail AVs flush + its 1/Z chain start early in block b;
        # the broadcast + normalize land a few k-steps later, and deferred
        # QKV/proj tiles are popped one per k-step to fill PE slack.
        def emit_norm_a(bid):
            # 1/Z chain, all off ACT: copy Z row out of psum, DMA-reshape to
            # [128,8] so the (slow, precise) DVE reciprocal runs 8 elems per
            # lane, DMA-reshape back.  un-copy frees the av psum banks.
            pair, s, av = blocks_done[bid]
            z_sb = sb.tile([1, 1024], F32, tag="z", bufs=2, name="z")
            nc.vector.tensor_copy(z_sb[:], av[64:65, :])
            zt = sb.tile([128, 8], F32, tag="zt", bufs=2, name="zt")
            nc.sync.dma_start(out=zt[:], in_=z_sb[:])
            un = sb.tile([64, 1024], B16, tag="un", bufs=2, name="un")
            nc.vector.tensor_copy(un[:], av[0:64, :])
            # write the reciprocal as f32r (same bits as f32) so the bc
            # matmul can read zr directly -- a bitcast AP here would break
            # the tile framework's dependency tracking (different tensor obj)
            ztr = sb.tile([128, 8], R32, tag="ztr", bufs=2, name="ztr")
            with nc.allow_low_precision(reason="softmax 1/Z"):
                nc.vector.reciprocal(ztr[:], zt[:])
            zr = sb.tile([1, 1024], R32, tag="zr", bufs=2, name="zr")
            nc.sync.dma_start(out=zr[:], in_=ztr[:])
            return (pair, s, un, zr)

        def emit_norm_b(norm_st):
            pair, s, un, zr = norm_st
            q0 = 512 * s
            # broadcast 1/Z to 64 partitions via K=1 rank-1 matmuls
            # (fp32 moving operand max is 512 wide -> one mm per half)
            bc = ps.tile([64, 1024], F32, tag="ps", bufs=2, name="bcps")
            mm(bc[:, 0:512], ones64[:], zr[:, 0:512], start=True, stop=True)
            mm(bc[:, 512:1024], ones64[:], zr[:, 512:1024], start=True, stop=True)
            nc.vector.tensor_tensor(
                attnT[pair][0:64, q0 : q0 + 512], un[:, 0:512], bc[:, 0:512], op=MUL
            )
            ntmp = sb.tile([64, 512], B16, tag="ntmp", bufs=2, name="ntmp")
            nc.vector.tensor_tensor(ntmp[:], un[:, 512:1024], bc[:, 512:1024], op=MUL)
            nc.sync.dma_start(out=attnT[pair][64:128, q0 : q0 + 512], in_=ntmp[:])

        pending = []  # (block_id, mm_args, mm_kwargs)
        blocks_done = {}  # bid -> (pair, s, av)
        fin_prev = None  # bid awaiting tail-flush + norm
        norm_st = None  # phase-A output awaiting broadcast+normalize
        deferred = []  # QKV/proj tile thunks, popped one per k-step
        blocks = [(p, s) for s in range(NS) for p in range(2)]
        for bid, (pair, s) in enumerate(blocks):
            if bid == 0:
                for mp in range(2):
                    emit_qkv_mp(0, mp)
                for jp in range(2):
                    emit_qkv_vp(0, jp)
            if pair == 0 and s + 1 < NS:
                deferred += [
                    lambda s=s, mp=mp: emit_qkv_mp(s + 1, mp) for mp in range(2)
                ]
                deferred += [
                    lambda s=s, jp=jp: emit_qkv_vp(s + 1, jp) for jp in range(2)
                ]
            q0 = 512 * s
            n_k = 4 * s + 4
            av = ps.tile([65, 1024], F32, tag="av", bufs=2, name="av")
            for k in range(n_k):
                k0 = 128 * k
                diag_t = k - 4 * s
                lo = 128 * diag_t if diag_t >= 0 else 0
                sp = ps.tile([128, 1024], F32, tag="ps", bufs=2)
                for h in range(2):
                    hb = slice(64 * h, 64 * h + 64)
                    off = 512 * h
                    mm(
                        sp[:, off + lo : off + 512],
                        kz[pair][hb, k0 : k0 + 128],
                        qT[pair][hb, q0 + lo : q0 + 512],
                        start=True,
                        stop=True,
                    )
                pt = sb.tile([128, 1024], B16, tag="pt", bufs=AV_DELAY + 2)
                if diag_t >= 0:
                    for h in range(2):
                        off = 512 * h
                        nc.scalar.activation(
                            pt[:, off + lo : off + 512],
                            sp[:, off + lo : off + 512],
                            EXP,
                            scale=SCALE,
                        )
                        nc.gpsimd.tensor_tensor(
                            pt[:, off + lo : off + lo + 128],
                            pt[:, off + lo : off + lo + 128],
                            tri[:],
                            op=MUL,
                        )
                else:
                    nc.scalar.activation(pt[:], sp[:], EXP, scale=SCALE)
                for h in range(2):
                    hg = 2 * pair + h
                    off = 512 * h
                    pending.append(
                        (
                            bid,
                            (
                                av[0:65, off + lo : off + 512],
                                vsb[k][:, 65 * hg : 65 * hg + 65],
                                pt[:, off + lo : off + 512],
                            ),
                            dict(
                                start=(k == 0),
                                stop=(k == n_k - 1),
                                skip_group_check=True,
                            ),
                        )
                    )
                    while len(pending) > AV_DELAY:
                        _, a, kw = pending.pop(0)
                        mm(*a, **kw)
                if k == 1 and fin_prev is not None:
                    # norm_b of the block BEFORE last: its 1/Z reshape chain
                    # (DVE + 2 small DMAs, ~5us latency) was started a full
                    # block ago, so its bc matmul no longer stalls the PE
                    if norm_st is not None:
                        npair, nspr = norm_st[0], norm_st[1]
                        emit_norm_b(norm_st)
                        norm_st = None
                        # proj(s) only after BOTH pairs' chunk-s normalizes
                        # are emitted -- popping any earlier would read attnT
                        # rows that have no producer yet
                        if npair == 1:
                            deferred += [
                                lambda s=nspr, mp=mp: emit_proj_mp(s, mp)
                                for mp in range(4)
                            ]
                    # independent PE work BEFORE the tail flush: the flushed
                    # AVs wait on the previous block's last exps (ACT is
                    # still catching up); these tiles keep the PE warm
                    for _ in range(2):
                        if deferred:
                            deferred.pop(0)()
                    pbid = fin_prev
                    while pending and pending[0][0] == pbid:
                        _, a, kw = pending.pop(0)
                        mm(*a, **kw)
                    norm_st = emit_norm_a(pbid)
                    fin_prev = None
                elif k >= 2 and deferred:
                    deferred.pop(0)()
            blocks_done[bid] = (pair, s, av)
            fin_prev = bid
        while pending:
            _, a, kw = pending.pop(0)
            mm(*a, **kw)
        last_st = emit_norm_a(fin_prev)
        if norm_st is not None:
            emit_norm_b(norm_st)
        while deferred:
            deferred.pop(0)()
        emit_norm_b(last_st)
        for mp in range(4):
            emit_proj_mp(NS - 1, mp)
    _split_multi_waits(nc)
    return nc


_NC_CACHE = None
LAST_RESULTS = None

_ONES = np.ones((1, 512), dtype=NPB16)
_ONES64 = np.ones((1, 64), dtype=np.float32)
_I, _J = np.meshgrid(np.arange(128), np.arange(128), indexing="ij")
_TRI01 = np.where(_J >= _I, 1.0, 0.0).astype(NPB16)


def _make_in_maps(x, Wqkv, bqkv, Wproj, bproj):
    in_maps = []
    for c in range(N_CORES):
        b, g = divmod(c, 4)
        qc = slice(CD * g, CD * (g + 1))
        wqc = Wqkv[:, qc]
        wk = Wqkv[:, D : 2 * D][:, qc]
        wv = Wqkv[:, 2 * D : 3 * D][:, qc]
        bq = bqkv[qc]
        bk = bqkv[D : 2 * D][qc]
        bvv = bqkv[2 * D : 3 * D][qc]
        # V columns interleaved per head: [wv_h (64 cols) | zero col]; the
        # ones-row rank-1 matmul adds [bv_h | 1.0] so Z rides along in AV.
        wv_i = np.zeros((D, VW), dtype=np.float32)
        bv_i = np.zeros((1, VW), dtype=np.float32)
        for h in range(HPC):
            wv_i[:, 65 * h : 65 * h + 64] = wv[:, 64 * h : 64 * h + 64]
            bv_i[0, 65 * h : 65 * h + 64] = bvv[64 * h : 64 * h + 64]
            bv_i[0, 65 * h + 64] = 1.0
        # host-blocked SBUF layouts (partition-major, fat contiguous lines)
        def kblock(a, width):
            # [1024, width] -> [128, 8*width] with col = width*k + c
            return a.reshape(N_DK, 128, width).transpose(1, 0, 2).reshape(128, -1)

        xT = x[b].T  # [D, L]
        xT_blk = (
            xT.reshape(N_DK, 128, NS, 512)
            .transpose(1, 2, 0, 3)
            .reshape(128, N_DK * L)
        )  # col = 4096*s + 512*k + t
        wq_blk = np.concatenate(
            [
                kblock(np.ascontiguousarray(wqc[:, 0:128]), 128),
                kblock(np.ascontiguousarray(wqc[:, 128:256]), 128),
                kblock(np.ascontiguousarray(wk[:, 0:128]), 128),
                kblock(np.ascontiguousarray(wk[:, 128:256]), 128),
                kblock(wv_i, VW),
            ],
            axis=1,
        )
        wproj_blk = (
            Wproj[qc, :].reshape(2, 128, D).transpose(1, 0, 2).reshape(128, 2 * D)
        )
        in_maps.append(
            {
                "xT": np.ascontiguousarray(xT_blk).astype(NPB16),
                "wqkv": np.ascontiguousarray(wq_blk).astype(NPB16),
                "bqk": np.concatenate([bq, bk]).reshape(1, 2 * CD).astype(NPB16),
                "bv": bv_i.astype(NPB16),
                "wproj": np.ascontiguousarray(wproj_blk).astype(NPB16),
                "onesr": _ONES,
                "ones64": _ONES64,
                "tri01": _TRI01,
            }
        )

    return in_maps


def kernel(x, Wqkv, bqkv, Wproj, bproj):
    global _NC_CACHE, LAST_RESULTS
    x = np.asarray(x, dtype=np.float32)
    Wqkv = np.asarray(Wqkv, dtype=np.float32)
    bqkv = np.asarray(bqkv, dtype=np.float32)
    Wproj = np.asarray(Wproj, dtype=np.float32)
    bproj = np.asarray(bproj, dtype=np.float32)

    if _NC_CACHE is None:
        _NC_CACHE = _build_program()
    nc = _NC_CACHE

    in_maps = _make_in_maps(x, Wqkv, bqkv, Wproj, bproj)
    res = run_bass_kernel_spmd(nc, in_maps, core_ids=list(range(N_CORES)))
    LAST_RESULTS = res

    out = np.empty((B, L, D), dtype=np.float32)
    for b in range(B):
        acc = res.results[4 * b]["yT"].astype(np.float32)
        for g in range(1, 4):
            acc = acc + res.results[4 * b + g]["yT"]
        out[b] = acc.T + bproj[None, :]
    return out
